# Initial kernel scaffold
#
"""DGCNN encoder Bass kernel for Trainium2 (8 NeuronCores, data-parallel over batch).

Algorithm notes (per core, one point cloud of N=2048 points):
  Each edge conv  y_i = max_{j in knn20(i)} bn_lrelu(W @ [x_j - x_i; x_i])
  is reformulated with A = W[:, :C], Cm = W[:, C:] - W[:, :C]:
      y_i = lrelu( s * (max_j (A x_j) + Cm x_i) + beta )       (s = gamma/sqrt(1+eps) > 0)
  so no per-edge features are ever materialized: Z = X A''^T (N x O) is computed
  once, the kNN top-20 row gather of Z happens via indirect DMA, and the max
  over neighbors commutes with the per-point additive term and the monotone
  bn_lrelu.  kNN ranking uses D = X X^T - 0.5*|x_j|^2 (row-rank-equivalent to
  the reference's -dist^2), top-20 per row via 3 rounds of DVE max8 /
  max_index / match_replace (exact).
"""

import sys

if "/opt/trn_rl_repo" not in sys.path:
    sys.path.insert(0, "/opt/trn_rl_repo")

from contextlib import ExitStack

import numpy as np

import concourse.bass as bass
import concourse.mybir as mybir
from concourse import bacc
from concourse.bass import IndirectOffsetOnAxis
from concourse.bass_utils import run_bass_kernel_spmd
from concourse.masks import make_identity
from concourse.tile import TileContext, add_dep_helper

EPS = 1e-5
K = 20
NEG_BIG = -3.0e38
F32 = mybir.dt.float32
U32 = mybir.dt.uint32
X_AX = mybir.AxisListType.X
COPY = mybir.ActivationFunctionType.Copy
SQUARE = mybir.ActivationFunctionType.Square
LRELU = mybir.ActivationFunctionType.Lrelu

# (C_in, O_out) per edge conv
CONVS = [(3, 64), (64, 64), (64, 128), (128, 256)]


def build_program(n_points=2048, max_conv=4, do_final=True, debug=False):
    """Build the single-core program (SPMD across 8 cores, batch-parallel)."""
    N = n_points
    T = N // 128  # row tiles
    JC = N // 512  # 512-wide column chunks of the distance matrix

    nc = bacc.Bacc(None, num_swdge_queues=4)

    pointsT = nc.declare_dram_parameter("pointsT", [3, N], F32, isOutput=False)
    # convs 1-3: acrhs carries the bias as a trailing row (paired with the
    # ones row of the augmented x tiles); conv4's input is a full 128
    # partitions so it keeps a separate bias matmul.
    acrhs = []
    for li, (C, O) in enumerate(CONVS):
        rows = C + 1 if li < 3 else C
        acrhs.append(
            nc.declare_dram_parameter(f"acrhs{li}", [rows, 2 * O], F32, isOutput=False)
        )
    acb4 = nc.declare_dram_parameter("acb3", [1, 2 * CONVS[3][1]], F32, isOutput=False)
    # w5c0/w5c1 carry the b5 bias / a zero row against x1T/x2T's ones rows.
    W5_SPLITS = [65, 65, 128, 128, 128]
    w5c = [
        nc.declare_dram_parameter(f"w5c{k}", [ck, 1024], F32, isOutput=False)
        for k, ck in enumerate(W5_SPLITS)
    ]
    out_t = nc.declare_dram_parameter("out", [1, 2 * 1024], F32, isOutput=True)
    if debug:
        dbg_ix = nc.declare_dram_parameter("dbg_ix", [N, 24], U32, isOutput=True)
        dbg_x1 = nc.declare_dram_parameter("dbg_x1", [64, N], F32, isOutputrue=True) if False else nc.declare_dram_parameter("dbg_x1", [64, N], F32, isOutput=True)
        dbg_hx = nc.declare_dram_parameter("dbg_hx", [1, N], F32, isOutput=True)
        dbg_z1 = nc.declare_dram_parameter("dbg_z1", [N, CONVS[0][1]], F32, isOutput=True)
        dbg_d0 = nc.declare_dram_parameter("dbg_d0", [128, N], F32, isOutput=True)
        dbg_g0 = nc.declare_dram_parameter("dbg_g0", [128, K * CONVS[0][1]], F32, isOutput=True)

    with ExitStack() as stack:
        tc = stack.enter_context(TileContext(nc))
        persist = stack.enter_context(tc.tile_pool(name="persist", bufs=1))
        dram = stack.enter_context(tc.tile_pool(name="dram", bufs=1, space="DRAM"))

        # ---- persistent SBUF state ----
        identity = persist.tile([128, 128], F32)
        make_identity(nc, identity[:])
        ones_row = persist.tile([1, N], F32)
        nc.gpsimd.memset(ones_row[:], 1.0)
        onescol = persist.tile([128, 1], F32)
        nc.gpsimd.memset(onescol[:], 1.0)
        # Two column-norm rows, ping-ponged per conv: conv li reads
        # hxbufs[li % 2]; conv li's backs incrementally write the NEXT conv's
        # norms into hxbufs[(li + 1) % 2], so only conv 0 runs a phase A.
        hxbufs = [
            persist.tile([1, N], F32, tag="hx0", name="hx0"),
            persist.tile([1, N], F32, tag="hx1", name="hx1"),
        ]

        # Conv inputs carry a trailing ONES row (where C < 128): the Z / W5
        # bias matmuls then fold into the main matmuls for free (matmul cost
        # is per output column; extra contraction rows are free). The ones
        # row also joins the distance matmul, adding a constant +1 to every
        # D entry — ranking-neutral.
        xt0 = persist.tile([4, N], F32)
        nc.gpsimd.memset(xt0[:], 1.0)
        nc.sync.dma_start(out=xt0[0:3, :], in_=pointsT[:])
        x1T = persist.tile([65, N], F32)
        nc.gpsimd.memset(x1T[64:65, :], 1.0)
        x2T = persist.tile([65, N], F32)
        nc.gpsimd.memset(x2T[64:65, :], 1.0)
        x3T = persist.tile([128, N], F32)
        x4Ta = persist.tile([128, N], F32)
        x4Tb = persist.tile([128, N], F32)

        acrhs_sb = []
        for li, (C, O) in enumerate(CONVS):
            rows = C + 1 if li < 3 else C
            a = persist.tile([rows, 2 * O], F32, tag=f"acrhs{li}")
            nc.sync.dma_start(out=a[:], in_=acrhs[li][:])
            acrhs_sb.append(a)
        acb4_sb = persist.tile([1, 2 * CONVS[3][1]], F32)
        nc.sync.dma_start(out=acb4_sb[:], in_=acb4[:])
        w5_sb = []
        for k, ck in enumerate(W5_SPLITS):
            w = persist.tile([ck, 1024], F32, tag=f"w5c{k}")
            nc.sync.dma_start(out=w[:], in_=w5c[k][:])
            w5_sb.append(w)

        maxacc = persist.tile([128, 1024], F32)
        sumacc = persist.tile([128, 1024], F32)
        maxp = persist.tile([128, 8], F32)
        avgp = persist.tile([1, 1024], F32)

        z_dram = [
            dram.tile([N, O], F32, tag=f"z{li}", name=f"z{li}")
            for li, (_, O) in enumerate(CONVS)
        ]

        conv_in = [xt0, x1T, x2T, x3T]
        conv_out = [[(x1T, 0)], [(x2T, 0)], [(x3T, 0)], [(x4Ta, 0), (x4Tb, 128)]]

        # Final-stage pools are opened for the whole conv region so the W5
        # matmul + pooling accumulation for row-tile m can run interleaved
        # with conv4's pipeline as soon as x4T[:, mc] lands.
        pfp = stack.enter_context(tc.tile_pool(name="pf", bufs=1, space="PSUM"))
        fsp = stack.enter_context(tc.tile_pool(name="fs", bufs=1))
        if do_final:
            nc.gpsimd.memset(maxacc[:], NEG_BIG)
            nc.gpsimd.memset(sumacc[:], 0.0)
        xks = [x1T, x2T, x3T, x4Ta, x4Tb]

        def emit_final_tile(m):
            mc = slice(m * 128, (m + 1) * 128)
            pf = pfp.tile([128, 1024], F32, space="PSUM", tag="pf",
                          name=f"pf_{m}")
            for h in range(2):
                hc = slice(h * 512, (h + 1) * 512)
                for k, xk in enumerate(xks):
                    nc.tensor.matmul(
                        pf[:, hc], xk[:, mc], w5_sb[k][:, hc],
                        start=(k == 0), stop=(k == len(xks) - 1),
                    )
            fs = fsp.tile([128, 1024], F32, tag="fs", name=f"fs_{m}")
            nc.scalar.activation(out=fs[:], in_=pf[:, :], func=COPY, scale=0.2)
            nc.vector.tensor_tensor(
                out=fs[:], in0=pf[:, :], in1=fs[:], op=mybir.AluOpType.max
            )
            nc.vector.tensor_tensor(
                out=maxacc[:], in0=maxacc[:], in1=fs[:], op=mybir.AluOpType.max
            )
            nc.vector.tensor_add(out=sumacc[:], in0=sumacc[:], in1=fs[:])

        for li, (C, O) in enumerate(CONVS):
            if li >= max_conv:
                break
            xT = conv_in[li]
            halfx2neg = hxbufs[li % 2]
            hxnext = hxbufs[(li + 1) % 2]
            with ExitStack() as cs:
                csbp = cs.enter_context(tc.tile_pool(name="csb", bufs=1))

                # ---- phase A (conv 0 only): -0.5 * |x_j|^2 from the points;
                # later convs get their norms incrementally from the previous
                # conv's backs. ----
                if li == 0:
                    with tc.tile_pool(name="px2", bufs=1, space="PSUM") as px2p, \
                            tc.tile_pool(name="sq", bufs=1) as sqp:
                        # Slice the ones row OUT of all distance-path math:
                        # even an exact +1 constant on D perturbs f32 rounding
                        # at the ulp scale that flips kNN boundary ties.
                        sq = sqp.tile([C, N], F32, tag="sq")
                        nc.scalar.activation(out=sq[:], in_=xT[0:C, :], func=SQUARE)
                        px2 = px2p.tile([1, N], F32, space="PSUM", tag="px2")
                        for j in range(JC):
                            jc = slice(j * 512, (j + 1) * 512)
                            nc.tensor.matmul(
                                px2[:, jc], onescol[0:C, :], sq[:, jc],
                                start=True, stop=True,
                            )
                        nc.scalar.activation(
                            out=halfx2neg[:, :], in_=px2[:, :], func=COPY,
                            scale=-0.5,
                        )
                        if debug:
                            nc.sync.dma_start(out=dbg_hx[:], in_=halfx2neg[:, :])

                z_writes = []
                needs_z_deps = []
                csb = csbp.tile([128, T, O], F32, tag="csb")
                with ExitStack() as ps:
                    pdp = ps.enter_context(
                        tc.tile_pool(name="pd", bufs=1, space="PSUM")
                    )
                    # ptrp is opened only after the Z loop's pzc pool closes:
                    # their lifetimes are disjoint and PSUM has room for only
                    # one of them next to pd (4 banks) + pf (2 banks).
                    ptrp = None
                    dsbp = ps.enter_context(tc.tile_pool(name="dsb", bufs=3))
                    vtp = ps.enter_context(tc.tile_pool(name="vt", bufs=3))
                    gtp = ps.enter_context(
                        tc.tile_pool(name="gt", bufs=3 if O <= 128 else 2)
                    )
                    smp = ps.enter_context(tc.tile_pool(name="sm", bufs=1))

                    # Software-pipelined emission: tile m's gather-consume
                    # (reduce/epilogue) is emitted AFTER tile m+1's top-k so
                    # the DVE works on tile m+1 while the Pool engine
                    # generates tile m's gather descriptors.
                    gsave = {}
                    ixsave = {}

                    def emit_self_dma(m, g):
                        mc = slice(m * 128, (m + 1) * 128)
                        # Issue from the Act queue: SP's in-order queue is
                        # clogged with z-write sem waits and would delay this.
                        sg = nc.scalar.dma_start(out=g[:, 0:O], in_=z_dram[li][mc, :])
                        for zw in z_writes:
                            add_dep_helper(
                                sg.ins, zw, sync=True,
                                reason="self-row read of z_dram (RAW)",
                            )

                    def emit_front_compute(m):
                        mc = slice(m * 128, (m + 1) * 128)
                        pd = pdp.tile([128, N], F32, space="PSUM", tag="pd",
                                      name=f"pd{li}_{m}")
                        for j in range(JC):
                            jc = slice(j * 512, (j + 1) * 512)
                            nc.tensor.matmul(
                                pd[:, jc], xT[0:C, mc], xT[0:C, jc],
                                start=True, stop=False,
                            )
                            nc.tensor.matmul(
                                pd[:, jc], ones_row[:, mc],
                                halfx2neg[:, jc],
                                start=False, stop=True,
                            )
                        dsb = dsbp.tile([128, N], F32, tag="dsb",
                                        name=f"dsb{li}_{m}")
                        nc.scalar.copy(out=dsb[:], in_=pd[:, :])

                        v = vtp.tile([128, 24], F32, tag="v", name=f"v{li}_{m}")
                        ix = vtp.tile([128, 24], U32, tag="ix", name=f"ix{li}_{m}")
                        # match_replace runs in place on dsb (it has no other
                        # readers after round 1), freeing the dz scratch so
                        # dsb can triple-buffer for the 3-deep pipeline.
                        nc.vector.max(out=v[:, 0:8], in_=dsb[:])
                        nc.vector.max_index(
                            out=ix[:, 0:8], in_max=v[:, 0:8], in_values=dsb[:]
                        )
                        nc.vector.match_replace(
                            out=dsb[:], in_to_replace=v[:, 0:8], in_values=dsb[:],
                            imm_value=NEG_BIG,
                        )
                        nc.vector.max(out=v[:, 8:16], in_=dsb[:])
                        nc.vector.max_index(
                            out=ix[:, 8:16], in_max=v[:, 8:16], in_values=dsb[:]
                        )
                        nc.vector.match_replace(
                            out=dsb[:], in_to_replace=v[:, 8:16], in_values=dsb[:],
                            imm_value=NEG_BIG,
                        )
                        nc.vector.max(out=v[:, 16:24], in_=dsb[:])
                        nc.vector.max_index(
                            out=ix[:, 16:24], in_max=v[:, 16:24], in_values=dsb[:]
                        )

                        if debug and li == 0:
                            nc.sync.dma_start(out=dbg_ix[mc, :], in_=ix[:, :])
                            if m == 0:
                                nc.sync.dma_start(out=dbg_d0[:, :], in_=dsb[:, :])
                        ixsave[m] = ix

                    def emit_front_gather(m):
                        ix = ixsave.pop(m)
                        g = gtp.tile([128, K * O], F32, tag="g", name=f"g{li}_{m}")
                        # Slot 0 is always the point itself (self-distance is
                        # the row max; a tie means an identical Z row), so it
                        # is a contiguous Z block -- fetch it with a plain
                        # HWDGE DMA instead of a Pool-engine indirect gather.
                        emit_self_dma(m, g)
                        for t in range(1, K):
                            gi = nc.gpsimd.indirect_dma_start(
                                out=g[:, t * O : (t + 1) * O],
                                out_offset=None,
                                in_=z_dram[li][:, :],
                                in_offset=IndirectOffsetOnAxis(
                                    ap=ix[:, t : t + 1], axis=0
                                ),
                            )
                            if m == 0 and t == 1:
                                # All gathers sit behind this one on the same
                                # SWDGE FIFO queue, so one sync edge per conv
                                # orders every gather after the Z writes.
                                for zw in z_writes:
                                    add_dep_helper(
                                        gi.ins, zw, sync=True,
                                        reason="gather reads z_dram (RAW)",
                                    )
                        gsave[m] = g

                    def emit_back(m):
                        mc = slice(m * 128, (m + 1) * 128)
                        g = gsave.pop(m)
                        O_ = O
                        # view gathered [128, K, O] as [128, O, K]; reduce over K
                        ga = g[:, :]
                        gview = bass.AP(
                            ga.tensor, ga.offset, [ga.ap[0], [1, O_], [O_, K]]
                        )
                        mx = smp.tile([128, O_], F32, tag="mx", name=f"mx{li}_{m}")
                        nc.vector.reduce_max(out=mx[:], in_=gview, axis=X_AX)
                        if debug and li == 0 and m == 0:
                            nc.sync.dma_start(out=dbg_g0[:, :], in_=g[:, :])

                        y = smp.tile([128, O_], F32, tag="y", name=f"y{li}_{m}")
                        nc.vector.tensor_add(out=y[:], in0=mx[:], in1=csb[:, m, :])
                        yl = smp.tile([128, O_], F32, tag="yl", name=f"yl{li}_{m}")
                        nc.scalar.activation(
                            out=yl[:], in_=y[:], func=COPY, scale=0.2
                        )
                        nc.vector.tensor_tensor(
                            out=yl[:], in0=y[:], in1=yl[:], op=mybir.AluOpType.max
                        )

                        for tgt, ocs in conv_out[li]:
                            w = min(128, O_ - ocs)
                            ptr = ptrp.tile([128, 128], F32, space="PSUM",
                                            tag="ptr", name=f"ptr{li}_{m}_{ocs}")
                            nc.tensor.transpose(
                                out=ptr[0:w, :], in_=yl[:, ocs : ocs + w],
                                identity=identity[:],
                            )
                            nc.scalar.copy(out=tgt[0:w, mc], in_=ptr[0:w, :])

                        if li < 3:
                            # Incrementally produce the NEXT conv's column
                            # norms for this tile: square the just-written
                            # feature-major slice, sum feature partitions via
                            # matmul, scale into hxnext. Reuses a ptr PSUM
                            # slot; the next conv then skips phase A.
                            tgt0 = conv_out[li][0][0]
                            sqc = smp.tile([O_, 128], F32, tag="sqc",
                                           name=f"sqc{li}_{m}")
                            nc.scalar.activation(
                                out=sqc[:], in_=tgt0[0:O_, mc], func=SQUARE
                            )
                            px2c = ptrp.tile([128, 128], F32, space="PSUM",
                                             tag="ptr", name=f"px2c{li}_{m}")
                            nc.tensor.matmul(
                                px2c[0:1, :], onescol[0:O_, :], sqc[:, :],
                                start=True, stop=True,
                            )
                            nc.scalar.activation(
                                out=hxnext[0:1, mc], in_=px2c[0:1, :],
                                func=COPY, scale=-0.5,
                            )

                    # Batched z writes: 16 serial SP DMAs cost ~2.1us each in
                    # SEQ+HWDGE+sem overhead and gated the conv's whole gather
                    # phase on their tail; stage Z in SBUF and write all 16
                    # row tiles in one DMA (conv4's staging doesn't fit and
                    # keeps per-tile writes).
                    # TB row tiles share one staging slot and one z DMA;
                    # conv4 staggers 4 batches through the same 8KB slot.
                    TB = T if O <= 128 else 4
                    zsb = csbp.tile([128, TB, O], F32, tag="zsb", name=f"zsb{li}")
                    zt = z_dram[li][:, :]
                    with ExitStack() as zs:
                        pzcp = zs.enter_context(
                            tc.tile_pool(name="pzc", bufs=2, space="PSUM")
                        )
                        for m in range(T):
                            mc = slice(m * 128, (m + 1) * 128)
                            pzc = pzcp.tile(
                                [128, 2 * O], F32, space="PSUM", tag="pzc"
                            )
                            if li < 3:
                                # bias folded into acrhs via the ones row
                                nc.tensor.matmul(
                                    pzc[:, :], xT[:, mc], acrhs_sb[li][:, :],
                                    start=True, stop=True,
                                )
                            else:
                                nc.tensor.matmul(
                                    pzc[:, :], xT[:, mc], acrhs_sb[li][:, :],
                                    start=True, stop=False,
                                )
                                nc.tensor.matmul(
                                    pzc[:, :], ones_row[:, mc], acb4_sb[:, :],
                                    start=False, stop=True,
                                )
                            nc.scalar.copy(out=zsb[:, m % TB, :], in_=pzc[:, 0:O])
                            nc.scalar.copy(out=csb[:, m, :], in_=pzc[:, O : 2 * O])
                            if m % TB == TB - 1:
                                zv = bass.AP(
                                    zt.tensor,
                                    (m // TB) * TB * 128 * O,
                                    [[O, 128], [128 * O, TB], [1, O]],
                                )
                                zw = nc.sync.dma_start(out=zv, in_=zsb[:, :, :])
                                z_writes.append(zw.ins)
                    # Tile 0's compute sits AFTER the Z loop: B's matmuls
                    # absorb the cold-p-state PE penalty off the critical
                    # chain and Z (the gather gate) finishes earlier; pd(0)
                    # runs at full clock right behind them.
                    emit_front_compute(0)
                    emit_front_gather(0)
                    ptrp = ps.enter_context(
                        tc.tile_pool(name="ptr", bufs=2, space="PSUM")
                    )

                    # 3-deep software pipeline: back(m) is emitted two fronts
                    # later so the in-order DVE queue never stalls at a
                    # reduce whose tail gathers are still in flight.
                    for m in range(1, T):
                        emit_front_compute(m)
                        emit_front_gather(m)
                        if m >= 2:
                            emit_back(m - 2)
                        if do_final and li == 3 and m >= 3:
                            emit_final_tile(m - 3)
                    # Drain tail: interleave the last W5/pooling tiles between
                    # the last backs so their DVE accumulates fill the gaps
                    # while the PE streams the remaining W5 matmuls.
                    emit_back(T - 2)
                    if do_final and li == 3:
                        emit_final_tile(T - 3)
                    emit_back(T - 1)
                    if do_final and li == 3:
                        emit_final_tile(T - 2)
                        emit_final_tile(T - 1)

        if not do_final:
            dummy = persist.tile([1, 2 * 1024], F32)
            nc.gpsimd.memset(dummy[:], 0.0)
            nc.sync.dma_start(out=out_t[:], in_=dummy[:, :])

        if debug:
            nc.sync.dma_start(out=dbg_x1[:], in_=x1T[:, :])
        if do_final:
            # ---- final epilogue: max+mean pool over N (W5 stage ran
            # interleaved with conv4 above) ----
            with ExitStack() as fs_stack:
                ptr2p = fs_stack.enter_context(
                    tc.tile_pool(name="ptr2", bufs=2, space="PSUM")
                )
                psp = fs_stack.enter_context(tc.tile_pool(name="ps", bufs=1, space="PSUM"))

                for c in range(8):
                    cc = slice(c * 128, (c + 1) * 128)
                    ptr2 = ptr2p.tile([128, 128], F32, space="PSUM", tag="ptr2")
                    nc.tensor.transpose(
                        out=ptr2[:], in_=maxacc[:, cc], identity=identity[:]
                    )
                    nc.vector.reduce_max(out=maxp[:, c : c + 1], in_=ptr2[:, :], axis=X_AX)
                psum_s = psp.tile([1, 1024], F32, space="PSUM", tag="psum_s")
                for h in range(2):
                    hc = slice(h * 512, (h + 1) * 512)
                    nc.tensor.matmul(
                        psum_s[:, hc], onescol[:, :], sumacc[:, hc],
                        start=True, stop=True,
                    )
                nc.scalar.activation(
                    out=avgp[:, :], in_=psum_s[:, :], func=COPY, scale=1.0 / N
                )

                outap = out_t[:]
                nc.sync.dma_start(
                    out=bass.AP(outap.tensor, 0, [[1, 128], [128, 8]]), in_=maxp[:, :]
                )
                nc.sync.dma_start(out=out_t[0:1, 1024:2048], in_=avgp[:, :])

    nc.finalize()
    return nc


def host_inputs(points, Ws, gs, bs, g5, b5, W5):
    """Host-side preprocessing -> per-core input maps (weights replicated)."""
    B = points.shape[0]
    shared = {}
    for li, (C, O) in enumerate(CONVS):
        W = np.asarray(Ws[li], np.float32)
        s = (np.asarray(gs[li], np.float32) / np.sqrt(np.float32(1.0 + EPS)))[:, None]
        A = (s * W[:, :C]).T.astype(np.float32)  # (C, O)
        Cm = (s * (W[:, C:] - W[:, :C])).T.astype(np.float32)  # (C, O)
        ac = np.ascontiguousarray(np.concatenate([A, Cm], axis=1), np.float32)
        biasrow = np.concatenate(
            [np.zeros((1, O), np.float32), np.asarray(bs[li], np.float32)[None, :]],
            axis=1,
        )
        if li < 3:
            # bias folded as a trailing row against the x tiles' ones row
            shared[f"acrhs{li}"] = np.ascontiguousarray(
                np.concatenate([ac, biasrow], axis=0), np.float32
            )
        else:
            shared[f"acrhs{li}"] = ac
            shared["acb3"] = biasrow
    s5 = (np.asarray(g5, np.float32) / np.sqrt(np.float32(1.0 + EPS)))[:, None]
    W5s = (s5 * np.asarray(W5, np.float32)).T.astype(np.float32)  # (512, 1024)
    b5r = np.asarray(b5, np.float32)[None, :]
    zrow = np.zeros((1, 1024), np.float32)
    # chunks follow [x1T(64)+ones, x2T(64)+ones, x3T(128), x4Ta, x4Tb]:
    # chunk 0 carries b5 against x1T's ones row, chunk 1 a zero row.
    chunks = [
        np.concatenate([W5s[0:64], b5r], axis=0),
        np.concatenate([W5s[64:128], zrow], axis=0),
        W5s[128:256],
        W5s[256:384],
        W5s[384:512],
    ]
    for k, w in enumerate(chunks):
        shared[f"w5c{k}"] = np.ascontiguousarray(w, np.float32)
    maps = []
    for b in range(B):
        m = dict(shared)
        m["pointsT"] = np.ascontiguousarray(
            np.asarray(points[b], np.float32).T, np.float32
        )
        maps.append(m)
    return maps


def _bust_stale_caches():
    # The libneuronxla NEFF cache key has been observed to collide across
    # different BIR payloads with identical HLO shapes, silently reusing a
    # stale NEFF.  A recompile is cheap insurance against wrong results.
    import shutil

    import glob
    import os

    dirs = [
        "/root/.neuron-compile-cache",
        "/tmp/no-user/neuroncc_compile_workdir",
        f"/tmp/neuron-compile-cache-uid{os.getuid()}",
    ] + glob.glob("/tmp/neuron-compile-cache-uid*")
    for d in dirs:
        shutil.rmtree(d, ignore_errors=True)


def kernel(points, W1, W2, W3, W4, W5, g1, g2, g3, g4, g5, b1, b2, b3, b4, b5):
    _bust_stale_caches()
    points = np.asarray(points, np.float32)
    B, N, _ = points.shape
    assert (B, N) == (8, 2048), (B, N)
    nc = build_program(N)
    in_maps = host_inputs(
        points, [W1, W2, W3, W4], [g1, g2, g3, g4], [b1, b2, b3, b4], g5, b5, W5
    )
    res = run_bass_kernel_spmd(nc, in_maps, list(range(8)))
    out = np.stack(
        [res.results[b]["out"].reshape(-1) for b in range(8)]
    ).astype(np.float32)
    return out



# revision 76
# speedup vs baseline: 1.0286x; 1.0286x over previous
"""DGCNN encoder Bass kernel for Trainium2 (8 NeuronCores, data-parallel over batch).

Algorithm notes (per core, one point cloud of N=2048 points):
  Each edge conv  y_i = max_{j in knn20(i)} bn_lrelu(W @ [x_j - x_i; x_i])
  is reformulated with A = W[:, :C], Cm = W[:, C:] - W[:, :C]:
      y_i = lrelu( s * (max_j (A x_j) + Cm x_i) + beta )       (s = gamma/sqrt(1+eps) > 0)
  so no per-edge features are ever materialized: Z = X A''^T (N x O) is computed
  once, the kNN top-20 row gather of Z happens via indirect DMA, and the max
  over neighbors commutes with the per-point additive term and the monotone
  bn_lrelu.  kNN ranking uses D = X X^T - 0.5*|x_j|^2 (row-rank-equivalent to
  the reference's -dist^2), top-20 per row via 3 rounds of DVE max8 /
  max_index / match_replace (exact).

  Optimization status (8 sessions): DVE-bound at ~91% on the exact top-k;
  hierarchical/gather-based restructurings are blocked by HW (one offset
  per partition in SWDGE indirect DMA; 256B dma_gather element floor; DMA
  CCE supports only add; fp16/quantized ranking breaks the 2e-2 gate).
  Next best unexplored lead: ship conv0's halfx2neg (-0.5|x|^2) from the
  host as pointsT row 4 (compute np.float32 stepwise as
  -0.5*((x*x+y*y)+z*z) to stay bitwise-identical to the Act-Square +
  PE-sum path) -- deletes phase A (~3us off the start chain). Needs a HW
  error check. Also: bump the nonce output shape per build (NEFF cache
  collides on HLO shapes).
"""

import sys

if "/opt/trn_rl_repo" not in sys.path:
    sys.path.insert(0, "/opt/trn_rl_repo")

from contextlib import ExitStack

import numpy as np

import concourse.bass as bass
import concourse.bass_isa as bass_isa
import concourse.mybir as mybir
from concourse import bacc
from concourse.bass import IndirectOffsetOnAxis
from concourse.bass_utils import run_bass_kernel_spmd
from concourse.masks import make_identity
from concourse.tile import TileContext, add_dep_helper

EPS = 1e-5
K = 20
NEG_BIG = -3.0e38
F32 = mybir.dt.float32
U32 = mybir.dt.uint32
X_AX = mybir.AxisListType.X
COPY = mybir.ActivationFunctionType.Copy
SQUARE = mybir.ActivationFunctionType.Square
LRELU = mybir.ActivationFunctionType.Lrelu

# (C_in, O_out) per edge conv
CONVS = [(3, 64), (64, 64), (64, 128), (128, 256)]


def build_program(n_points=2048, max_conv=4, do_final=True, debug=False):
    """Build the single-core program (SPMD across 8 cores, batch-parallel)."""
    N = n_points
    T = N // 128  # row tiles
    JC = N // 512  # 512-wide column chunks of the distance matrix

    nc = bacc.Bacc(None, num_swdge_queues=4)

    pointsT = nc.declare_dram_parameter("pointsT", [4, N], F32, isOutput=False)
    # convs 1-3: acrhs carries the bias as a trailing row (paired with the
    # ones row of the augmented x tiles); conv4's input is a full 128
    # partitions so it keeps a separate bias matmul.
    acrhs = []
    for li, (C, O) in enumerate(CONVS):
        rows = C + 1 if li < 3 else C
        acrhs.append(
            nc.declare_dram_parameter(f"acrhs{li}", [rows, 2 * O], F32, isOutput=False)
        )
    acb4 = nc.declare_dram_parameter("acb3", [1, 2 * CONVS[3][1]], F32, isOutput=False)
    # w5c0/w5c1 carry the b5 bias / a zero row against x1T/x2T's ones rows.
    W5_SPLITS = [65, 65, 128, 128, 128]
    w5c = [
        nc.declare_dram_parameter(f"w5c{k}", [ck, 1024], F32, isOutput=False)
        for k, ck in enumerate(W5_SPLITS)
    ]
    out_t = nc.declare_dram_parameter("out", [1, 2 * 1024], F32, isOutput=True)
    # The libneuronxla NEFF cache keys on HLO shapes and has been observed to
    # collide across different BIR payloads; a build-specific output shape
    # makes this program's HLO unique so a stale NEFF can never be reused.
    nonce_t = nc.declare_dram_parameter("nonce", [1, 67], F32, isOutput=True)
    if debug:
        dbg_ix = nc.declare_dram_parameter("dbg_ix", [N, 24], U32, isOutput=True)
        dbg_x1 = nc.declare_dram_parameter("dbg_x1", [64, N], F32, isOutputrue=True) if False else nc.declare_dram_parameter("dbg_x1", [64, N], F32, isOutput=True)
        dbg_hx = nc.declare_dram_parameter("dbg_hx", [1, N], F32, isOutput=True)
        dbg_z1 = nc.declare_dram_parameter("dbg_z1", [N, CONVS[0][1]], F32, isOutput=True)
        dbg_d0 = nc.declare_dram_parameter("dbg_d0", [128, N], F32, isOutput=True)
        dbg_g0 = nc.declare_dram_parameter("dbg_g0", [128, K * CONVS[0][1]], F32, isOutput=True)

    with ExitStack() as stack:
        tc = stack.enter_context(TileContext(nc))
        persist = stack.enter_context(tc.tile_pool(name="persist", bufs=1))
        dram = stack.enter_context(tc.tile_pool(name="dram", bufs=1, space="DRAM"))

        # ---- persistent SBUF state ----
        identity = persist.tile([128, 128], F32)
        make_identity(nc, identity[:])
        ones_row = persist.tile([1, N], F32)
        nc.gpsimd.memset(ones_row[:], 1.0)
        onescol = persist.tile([128, 1], F32)
        nc.gpsimd.memset(onescol[:], 1.0)
        # Two column-norm rows, ping-ponged per conv: conv li reads
        # hxbufs[li % 2]; conv li's backs incrementally write the NEXT conv's
        # norms into hxbufs[(li + 1) % 2], so only conv 0 runs a phase A.
        hxbufs = [
            persist.tile([1, N], F32, tag="hx0", name="hx0"),
            persist.tile([1, N], F32, tag="hx1", name="hx1"),
        ]

        # Conv inputs carry a trailing ONES row (where C < 128): the Z / W5
        # bias matmuls then fold into the main matmuls for free (matmul cost
        # is per output column; extra contraction rows are free). The ones
        # row also joins the distance matmul, adding a constant +1 to every
        # D entry — ranking-neutral.
        xt0 = persist.tile([4, N], F32)
        # the host ships [points.T ; ones] as one [4, N] tensor: a single
        # DMA with no memset ahead of it (a full-tile memset would gate the
        # points DMA with a WAW hazard).
        nc.sync.dma_start(out=xt0[0:4, :], in_=pointsT[:])
        x1T = persist.tile([65, N], F32)
        nc.gpsimd.memset(x1T[64:65, :], 1.0)
        x2T = persist.tile([65, N], F32)
        nc.gpsimd.memset(x2T[64:65, :], 1.0)
        x3T = persist.tile([128, N], F32)
        x4Ta = persist.tile([128, N], F32)
        x4Tb = persist.tile([128, N], F32)

        acrhs_sb = []
        for li, (C, O) in enumerate(CONVS):
            rows = C + 1 if li < 3 else C
            a = persist.tile([rows, 2 * O], F32, tag=f"acrhs{li}")
            nc.sync.dma_start(out=a[:], in_=acrhs[li][:])
            acrhs_sb.append(a)
        acb4_sb = persist.tile([1, 2 * CONVS[3][1]], F32)
        nc.sync.dma_start(out=acb4_sb[:], in_=acb4[:])
        w5_sb = []
        for k, ck in enumerate(W5_SPLITS):
            w = persist.tile([ck, 1024], F32, tag=f"w5c{k}")
            nc.sync.dma_start(out=w[:], in_=w5c[k][:])
            w5_sb.append(w)

        # PE warmup: the first ~3us of matmuls run 2-3.7x slower while the
        # PE ramps p-states; burn that ramp on dummy transposes during the
        # initial weight/points DMAs instead of the first dist matmuls.
        with tc.tile_pool(name="warm", bufs=1, space="PSUM") as warmp:
            wt = warmp.tile([128, 128], F32, space="PSUM", tag="warm")
            for _ in range(40):
                nc.tensor.transpose(
                    out=wt[:], in_=identity[:], identity=identity[:]
                )

        noncebuf = persist.tile([1, 67], F32)
        nc.gpsimd.memset(noncebuf[:], 0.0)
        nc.sync.dma_start(out=nonce_t[:], in_=noncebuf[:, :])

        maxacc = persist.tile([128, 1024], F32)
        sumacc = persist.tile([128, 1024], F32)

        z_dram = [
            dram.tile([N, O], F32, tag=f"z{li}", name=f"z{li}")
            for li, (_, O) in enumerate(CONVS)
        ]

        conv_in = [xt0, x1T, x2T, x3T]
        conv_out = [[(x1T, 0)], [(x2T, 0)], [(x3T, 0)], [(x4Ta, 0), (x4Tb, 128)]]

        # Cross-conv Z overlap: conv li+1's Z matmul for tile m only needs
        # x_{li+1}[:, mc], which conv li's back(m) has just written -- so the
        # whole next-conv Z phase runs interleaved with the current conv's
        # pipeline and the conv-boundary Z-loop wait disappears. csb/zsb are
        # persistent per conv; one shared PSUM pool (2 banks) serves every
        # conv's Z matmuls (it is idle exactly when conv4's pf pool runs).
        # csb/zsb for conv li live from conv li-1's pipeline (where zemit
        # fills them) to the end of conv li: a bufs=2 pool with tag cycling
        # gives exactly that two-conv lifetime, sized by the largest conv.
        TB_all = [4 if O2 <= 128 else 2 for _, O2 in CONVS]
        csb_all = [None] * len(CONVS)
        zsb_all = [None] * len(CONVS)
        csbzp = stack.enter_context(tc.tile_pool(name="csbz", bufs=2))

        def open_csb(li2):
            C2, O2 = CONVS[li2]
            csb_all[li2] = csbzp.tile([128, T, O2], F32, tag="csb",
                                      name=f"csb{li2}")
            zsb_all[li2] = csbzp.tile([128, TB_all[li2], O2], F32,
                                      tag="zsb", name=f"zsb{li2}")

        z_writes_all = [[] for _ in CONVS]
        # pzcp's 2 PSUM banks are returned after conv3's Z (emitted during
        # conv li==2) so conv4's pf pool can take them over.
        pz_es = ExitStack()
        pzcp = pz_es.enter_context(tc.tile_pool(name="pzc", bufs=2, space="PSUM"))

        def zemit(li2, m):
            """Z + csb matmul for conv li2, tile m (batched DRAM write)."""
            C2, O2 = CONVS[li2]
            TB2 = TB_all[li2]
            xT2 = conv_in[li2]
            mc = slice(m * 128, (m + 1) * 128)
            pzc = pzcp.tile([128, 2 * O2], F32, space="PSUM", tag="pzc")
            if li2 < 3:
                # bias folded into acrhs via the ones row
                nc.tensor.matmul(
                    pzc[:, :], xT2[:, mc], acrhs_sb[li2][:, :],
                    start=True, stop=True,
                )
            else:
                # acb3's Z half (cols 0:O) is all zeros: the bias only feeds
                # the csb half, so the bias matmul runs at half width.
                nc.tensor.matmul(
                    pzc[:, :], xT2[:, mc], acrhs_sb[li2][:, :],
                    start=True, stop=False,
                )
                nc.tensor.matmul(
                    pzc[:, O2 : 2 * O2], ones_row[:, mc],
                    acb4_sb[:, O2 : 2 * O2],
                    start=False, stop=True,
                    skip_group_check=True,
                )
            nc.scalar.copy(out=zsb_all[li2][:, m % TB2, :], in_=pzc[:, 0:O2])
            nc.scalar.copy(out=csb_all[li2][:, m, :], in_=pzc[:, O2 : 2 * O2])
            if m % TB2 == TB2 - 1:
                zt2 = z_dram[li2][:, :]
                zv = bass.AP(
                    zt2.tensor,
                    (m // TB2) * TB2 * 128 * O2,
                    [[O2, 128], [128 * O2, TB2], [1, O2]],
                )
                zw = nc.sync.dma_start(out=zv, in_=zsb_all[li2][:, :, :])
                z_writes_all[li2].append(zw.ins)

        # Final-stage pools: fs (SBUF) up front; pf (PSUM) lazily at conv4,
        # after pzcp's banks are returned.
        pf_holder = {}
        fsp = stack.enter_context(tc.tile_pool(name="fs", bufs=1))
        if do_final:
            nc.gpsimd.memset(maxacc[:], NEG_BIG)
            nc.gpsimd.memset(sumacc[:], 0.0)
        xks = [x1T, x2T, x3T, x4Ta, x4Tb]

        def emit_final_tile(m):
            pfp = pf_holder["pfp"]
            mc = slice(m * 128, (m + 1) * 128)
            pf = pfp.tile([128, 1024], F32, space="PSUM", tag="pf",
                          name=f"pf_{m}")
            for h in range(2):
                hc = slice(h * 512, (h + 1) * 512)
                for k, xk in enumerate(xks):
                    nc.tensor.matmul(
                        pf[:, hc], xk[:, mc], w5_sb[k][:, hc],
                        start=(k == 0), stop=(k == len(xks) - 1),
                    )
            fs = fsp.tile([128, 1024], F32, tag="fs", name=f"fs_{m}")
            nc.scalar.activation(out=fs[:], in_=pf[:, :], func=COPY, scale=0.2)
            nc.vector.tensor_tensor(
                out=fs[:], in0=pf[:, :], in1=fs[:], op=mybir.AluOpType.max
            )
            nc.vector.tensor_tensor(
                out=maxacc[:], in0=maxacc[:], in1=fs[:], op=mybir.AluOpType.max
            )
            nc.vector.tensor_add(out=sumacc[:], in0=sumacc[:], in1=fs[:])

        for li, (C, O) in enumerate(CONVS):
            if li >= max_conv:
                break
            if li == 0:
                open_csb(0)
            if li < 3:
                open_csb(li + 1)  # filled by this conv's zemit calls
            if li == 3:
                # all Z phases are emitted; free pzc's banks for pf
                pz_es.close()
                pf_holder["pfp"] = stack.enter_context(
                    tc.tile_pool(name="pf", bufs=1, space="PSUM")
                )
            xT = conv_in[li]
            halfx2neg = hxbufs[li % 2]
            hxnext = hxbufs[(li + 1) % 2]
            with ExitStack() as cs:

                # ---- phase A (conv 0 only): -0.5 * |x_j|^2 from the points;
                # later convs get their norms incrementally from the previous
                # conv's backs. ----
                if li == 0:
                    with tc.tile_pool(name="px2", bufs=1, space="PSUM") as px2p, \
                            tc.tile_pool(name="sq", bufs=1) as sqp:
                        # Slice the ones row OUT of all distance-path math:
                        # even an exact +1 constant on D perturbs f32 rounding
                        # at the ulp scale that flips kNN boundary ties.
                        sq = sqp.tile([C, N], F32, tag="sq")
                        nc.scalar.activation(out=sq[:], in_=xT[0:C, :], func=SQUARE)
                        px2 = px2p.tile([1, N], F32, space="PSUM", tag="px2")
                        for j in range(JC):
                            jc = slice(j * 512, (j + 1) * 512)
                            nc.tensor.matmul(
                                px2[:, jc], onescol[0:C, :], sq[:, jc],
                                start=True, stop=True,
                            )
                        nc.scalar.activation(
                            out=halfx2neg[:, :], in_=px2[:, :], func=COPY,
                            scale=-0.5,
                        )
                        if debug:
                            nc.sync.dma_start(out=dbg_hx[:], in_=halfx2neg[:, :])

                needs_z_deps = []
                with ExitStack() as ps:
                    pdp = ps.enter_context(
                        tc.tile_pool(name="pd", bufs=1, space="PSUM")
                    )
                    # ptrp is opened only after the Z loop's pzc pool closes:
                    # their lifetimes are disjoint and PSUM has room for only
                    # one of them next to pd (4 banks) + pf (2 banks).
                    ptrp = None
                    dsbp = ps.enter_context(
                        tc.tile_pool(name="dsb", bufs=3 if O <= 128 else 2)
                    )
                    vtp = ps.enter_context(tc.tile_pool(name="vt", bufs=3))
                    gtp = ps.enter_context(
                        tc.tile_pool(name="gt", bufs=3 if O <= 128 else 2)
                    )
                    smp = ps.enter_context(tc.tile_pool(name="sm", bufs=1))

                    # Software-pipelined emission: tile m's gather-consume
                    # (reduce/epilogue) is emitted AFTER tile m+1's top-k so
                    # the DVE works on tile m+1 while the Pool engine
                    # generates tile m's gather descriptors.
                    gsave = {}
                    ixsave = {}

                    def emit_self_dma(m, g):
                        mc = slice(m * 128, (m + 1) * 128)
                        # Issue from the Act queue: SP's in-order queue is
                        # clogged with z-write sem waits and would delay this.
                        sg = nc.scalar.dma_start(out=g[:, 0:O], in_=z_dram[li][mc, :])
                        for zw in z_writes_all[li]:
                            add_dep_helper(
                                sg.ins, zw, sync=True,
                                reason="self-row read of z_dram (RAW)",
                            )

                    def emit_front_compute(m):
                        mc = slice(m * 128, (m + 1) * 128)
                        pd = pdp.tile([128, N], F32, space="PSUM", tag="pd",
                                      name=f"pd{li}_{m}")
                        for j in range(JC):
                            jc = slice(j * 512, (j + 1) * 512)
                            nc.tensor.matmul(
                                pd[:, jc], xT[0:C, mc], xT[0:C, jc],
                                start=True, stop=False,
                            )
                            nc.tensor.matmul(
                                pd[:, jc], ones_row[:, mc],
                                halfx2neg[:, jc],
                                start=False, stop=True,
                            )
                        dsb = dsbp.tile([128, N], F32, tag="dsb",
                                        name=f"dsb{li}_{m}")
                        nc.scalar.copy(out=dsb[:], in_=pd[:, :])

                        v = vtp.tile([128, 24], F32, tag="v", name=f"v{li}_{m}")
                        ix = vtp.tile([128, 24], U32, tag="ix", name=f"ix{li}_{m}")
                        # match_replace runs in place on dsb (it has no other
                        # readers after round 1), freeing the dz scratch so
                        # dsb can triple-buffer for the 3-deep pipeline.
                        nc.vector.max(out=v[:, 0:8], in_=dsb[:])
                        nc.vector.max_index(
                            out=ix[:, 0:8], in_max=v[:, 0:8], in_values=dsb[:]
                        )
                        nc.vector.match_replace(
                            out=dsb[:], in_to_replace=v[:, 0:8], in_values=dsb[:],
                            imm_value=NEG_BIG,
                        )
                        nc.vector.max(out=v[:, 8:16], in_=dsb[:])
                        nc.vector.max_index(
                            out=ix[:, 8:16], in_max=v[:, 8:16], in_values=dsb[:]
                        )
                        nc.vector.match_replace(
                            out=dsb[:], in_to_replace=v[:, 8:16], in_values=dsb[:],
                            imm_value=NEG_BIG,
                        )
                        nc.vector.max(out=v[:, 16:24], in_=dsb[:])
                        nc.vector.max_index(
                            out=ix[:, 16:24], in_max=v[:, 16:24], in_values=dsb[:]
                        )

                        if debug and li == 0:
                            nc.sync.dma_start(out=dbg_ix[mc, :], in_=ix[:, :])
                            if m == 0:
                                nc.sync.dma_start(out=dbg_d0[:, :], in_=dsb[:, :])
                        ixsave[m] = ix

                    def emit_front_gather(m):
                        ix = ixsave.pop(m)
                        g = gtp.tile([128, K * O], F32, tag="g", name=f"g{li}_{m}")
                        # Slot 0 is always the point itself (self-distance is
                        # the row max; a tie means an identical Z row), so it
                        # is a contiguous Z block -- fetch it with a plain
                        # HWDGE DMA instead of a Pool-engine indirect gather.
                        emit_self_dma(m, g)
                        for t in range(1, K):
                            gi = nc.gpsimd.indirect_dma_start(
                                out=g[:, t * O : (t + 1) * O],
                                out_offset=None,
                                in_=z_dram[li][:, :],
                                in_offset=IndirectOffsetOnAxis(
                                    ap=ix[:, t : t + 1], axis=0
                                ),
                            )
                            if m == 0 and t == 1:
                                # All gathers sit behind this one on the same
                                # SWDGE FIFO queue, so one sync edge per conv
                                # orders every gather after the Z writes.
                                for zw in z_writes_all[li]:
                                    add_dep_helper(
                                        gi.ins, zw, sync=True,
                                        reason="gather reads z_dram (RAW)",
                                    )
                        gsave[m] = g

                    def emit_back(m):
                        mc = slice(m * 128, (m + 1) * 128)
                        g = gsave.pop(m)
                        O_ = O
                        # view gathered [128, K, O] as [128, O, K]; reduce over K
                        ga = g[:, :]
                        mx = smp.tile([128, O_], F32, tag="mx", name=f"mx{li}_{m}")
                        # split the K-reduce so the first half can issue as
                        # soon as the first 10 gathered slots land (shorter
                        # fill bubbles at conv starts); max is associative.
                        KH = K // 2
                        gva = bass.AP(
                            ga.tensor, ga.offset, [ga.ap[0], [1, O_], [O_, KH]]
                        )
                        gvb = bass.AP(
                            ga.tensor, ga.offset + KH * O_,
                            [ga.ap[0], [1, O_], [O_, K - KH]],
                        )
                        mxb = smp.tile([128, O_], F32, tag="mxb",
                                       name=f"mxb{li}_{m}")
                        nc.vector.reduce_max(out=mx[:], in_=gva, axis=X_AX)
                        nc.vector.reduce_max(out=mxb[:], in_=gvb, axis=X_AX)
                        nc.vector.tensor_tensor(
                            out=mx[:], in0=mx[:], in1=mxb[:], op=mybir.AluOpType.max
                        )
                        if debug and li == 0 and m == 0:
                            nc.sync.dma_start(out=dbg_g0[:, :], in_=g[:, :])

                        y = smp.tile([128, O_], F32, tag="y", name=f"y{li}_{m}")
                        nc.vector.tensor_add(
                            out=y[:], in0=mx[:], in1=csb_all[li][:, m, :]
                        )
                        yl = smp.tile([128, O_], F32, tag="yl", name=f"yl{li}_{m}")
                        nc.scalar.activation(
                            out=yl[:], in_=y[:], func=COPY, scale=0.2
                        )
                        nc.vector.tensor_tensor(
                            out=yl[:], in0=y[:], in1=yl[:], op=mybir.AluOpType.max
                        )

                        for tgt, ocs in conv_out[li]:
                            w = min(128, O_ - ocs)
                            ptr = ptrp.tile([128, 128], F32, space="PSUM",
                                            tag="ptr", name=f"ptr{li}_{m}_{ocs}")
                            nc.tensor.transpose(
                                out=ptr[0:w, :], in_=yl[:, ocs : ocs + w],
                                identity=identity[:],
                            )
                            nc.scalar.copy(out=tgt[0:w, mc], in_=ptr[0:w, :])

                        if li < 3:
                            # Incrementally produce the NEXT conv's column
                            # norms for this tile: square the just-written
                            # feature-major slice, sum feature partitions via
                            # matmul, scale into hxnext. Reuses a ptr PSUM
                            # slot; the next conv then skips phase A.
                            tgt0 = conv_out[li][0][0]
                            sqc = smp.tile([O_, 128], F32, tag="sqc",
                                           name=f"sqc{li}_{m}")
                            nc.scalar.activation(
                                out=sqc[:], in_=tgt0[0:O_, mc], func=SQUARE
                            )
                            px2c = ptrp.tile([128, 128], F32, space="PSUM",
                                             tag="ptr", name=f"px2c{li}_{m}")
                            nc.tensor.matmul(
                                px2c[0:1, :], onescol[0:O_, :], sqc[:, :],
                                start=True, stop=True,
                            )
                            nc.scalar.activation(
                                out=hxnext[0:1, mc], in_=px2c[0:1, :],
                                func=COPY, scale=-0.5,
                            )

                    # Batched z writes: 16 serial SP DMAs cost ~2.1us each in
                    # SEQ+HWDGE+sem overhead and gated the conv's whole gather
                    # phase on their tail; stage Z in SBUF and write all 16
                    # row tiles in one DMA (conv4's staging doesn't fit and
                    # Conv 0's Z phase runs inline (no previous conv to
                    # hide it under); convs 1-3 had theirs emitted during
                    # the previous conv's pipeline, so they start on the
                    # distance matmuls immediately.
                    if li == 0:
                        for m in range(T):
                            zemit(0, m)
                    emit_front_compute(0)
                    if li in (1, 2):
                        # the previous conv's drain deferred these two so the
                        # PE reached dist(0) without zemits in between; the
                        # z-write tail still precedes gather(0)'s issue.
                        zemit(li, T - 2)
                        zemit(li, T - 1)
                    emit_front_gather(0)
                    ptrp = ps.enter_context(
                        tc.tile_pool(name="ptr", bufs=2, space="PSUM")
                    )

                    # 3-deep software pipeline: back(m) is emitted two fronts
                    # later so the in-order DVE queue never stalls at a
                    # reduce whose tail gathers are still in flight. The NEXT
                    # conv's Z matmul for tile m rides right behind back(m).
                    DL = 2
                    for m in range(1, T):
                        emit_front_compute(m)
                        emit_front_gather(m)
                        if m >= DL:
                            emit_back(m - DL)
                            if li < 3:
                                zemit(li + 1, m - DL)
                        if do_final and li == 3 and m >= 3:
                            emit_final_tile(m - 3)
                    # Drain tail: interleave the last W5/pooling tiles between
                    # the last backs so their DVE accumulates fill the gaps
                    # while the PE streams the remaining W5 matmuls.
                    for mm in range(T - DL, T - 2):
                        emit_back(mm)
                        if li < 3:
                            zemit(li + 1, mm)
                    emit_back(T - 2)
                    if li == 2:
                        zemit(3, T - 2)
                    if do_final and li == 3:
                        emit_final_tile(T - 3)
                    emit_back(T - 1)
                    if li == 2:
                        zemit(3, T - 1)
                    if do_final and li == 3:
                        emit_final_tile(T - 2)
                        emit_final_tile(T - 1)

        if not do_final:
            dummy = persist.tile([1, 2 * 1024], F32)
            nc.gpsimd.memset(dummy[:], 0.0)
            nc.sync.dma_start(out=out_t[:], in_=dummy[:, :])

        if debug:
            nc.sync.dma_start(out=dbg_x1[:], in_=x1T[:, :])
        if do_final:
            # ---- final epilogue: max+mean pool over N (W5 stage ran
            # interleaved with conv4 above) ----
            with ExitStack() as fs_stack:
                psp = fs_stack.enter_context(tc.tile_pool(name="ps", bufs=1, space="PSUM"))
                poolp = fs_stack.enter_context(tc.tile_pool(name="poolo", bufs=1))
                allr = poolp.tile([128, 1024], F32)
                avgp = poolp.tile([1, 1024], F32)

                # max over the point axis = partition reduce: one GPSIMD
                # all-reduce replaces 8 PE-transpose + DVE-reduce pairs.
                nc.gpsimd.partition_all_reduce(
                    allr[:], maxacc[:], channels=128,
                    reduce_op=bass_isa.ReduceOp.max,
                )
                psum_s = psp.tile([1, 1024], F32, space="PSUM", tag="psum_s")
                for h in range(2):
                    hc = slice(h * 512, (h + 1) * 512)
                    nc.tensor.matmul(
                        psum_s[:, hc], onescol[:, :], sumacc[:, hc],
                        start=True, stop=True,
                    )
                nc.scalar.activation(
                    out=avgp[:, :], in_=psum_s[:, :], func=COPY, scale=1.0 / N
                )

                nc.sync.dma_start(out=out_t[0:1, 0:1024], in_=allr[0:1, :])
                nc.sync.dma_start(out=out_t[0:1, 1024:2048], in_=avgp[:, :])

    nc.finalize()
    return nc


def host_inputs(points, Ws, gs, bs, g5, b5, W5):
    """Host-side preprocessing -> per-core input maps (weights replicated)."""
    B = points.shape[0]
    shared = {}
    for li, (C, O) in enumerate(CONVS):
        W = np.asarray(Ws[li], np.float32)
        s = (np.asarray(gs[li], np.float32) / np.sqrt(np.float32(1.0 + EPS)))[:, None]
        A = (s * W[:, :C]).T.astype(np.float32)  # (C, O)
        Cm = (s * (W[:, C:] - W[:, :C])).T.astype(np.float32)  # (C, O)
        ac = np.ascontiguousarray(np.concatenate([A, Cm], axis=1), np.float32)
        biasrow = np.concatenate(
            [np.zeros((1, O), np.float32), np.asarray(bs[li], np.float32)[None, :]],
            axis=1,
        )
        if li < 3:
            # bias folded as a trailing row against the x tiles' ones row
            shared[f"acrhs{li}"] = np.ascontiguousarray(
                np.concatenate([ac, biasrow], axis=0), np.float32
            )
        else:
            shared[f"acrhs{li}"] = ac
            shared["acb3"] = biasrow
    s5 = (np.asarray(g5, np.float32) / np.sqrt(np.float32(1.0 + EPS)))[:, None]
    W5s = (s5 * np.asarray(W5, np.float32)).T.astype(np.float32)  # (512, 1024)
    b5r = np.asarray(b5, np.float32)[None, :]
    zrow = np.zeros((1, 1024), np.float32)
    # chunks follow [x1T(64)+ones, x2T(64)+ones, x3T(128), x4Ta, x4Tb]:
    # chunk 0 carries b5 against x1T's ones row, chunk 1 a zero row.
    chunks = [
        np.concatenate([W5s[0:64], b5r], axis=0),
        np.concatenate([W5s[64:128], zrow], axis=0),
        W5s[128:256],
        W5s[256:384],
        W5s[384:512],
    ]
    for k, w in enumerate(chunks):
        shared[f"w5c{k}"] = np.ascontiguousarray(w, np.float32)
    maps = []
    for b in range(B):
        m = dict(shared)
        pt = np.asarray(points[b], np.float32).T  # (3, N)
        m["pointsT"] = np.ascontiguousarray(
            np.concatenate([pt, np.ones((1, pt.shape[1]), np.float32)], axis=0)
        )
        maps.append(m)
    return maps


def _bust_stale_caches():
    # The libneuronxla NEFF cache key has been observed to collide across
    # different BIR payloads with identical HLO shapes, silently reusing a
    # stale NEFF.  A recompile is cheap insurance against wrong results.
    import shutil

    import glob
    import os

    dirs = [
        "/root/.neuron-compile-cache",
        "/tmp/no-user/neuroncc_compile_workdir",
        f"/tmp/neuron-compile-cache-uid{os.getuid()}",
    ] + glob.glob("/tmp/neuron-compile-cache-uid*")
    for d in dirs:
        shutil.rmtree(d, ignore_errors=True)


def kernel(points, W1, W2, W3, W4, W5, g1, g2, g3, g4, g5, b1, b2, b3, b4, b5):
    _bust_stale_caches()
    points = np.asarray(points, np.float32)
    B, N, _ = points.shape
    assert (B, N) == (8, 2048), (B, N)
    nc = build_program(N)
    in_maps = host_inputs(
        points, [W1, W2, W3, W4], [g1, g2, g3, g4], [b1, b2, b3, b4], g5, b5, W5
    )
    res = run_bass_kernel_spmd(nc, in_maps, list(range(8)))
    out = np.stack(
        [res.results[b]["out"].reshape(-1) for b in range(8)]
    ).astype(np.float32)
    return out

